# revision 31
# baseline (speedup 1.0000x reference)
"""Distributed Trainium2 kernel for nn_Attention (dense transformer block:
fused QKV projection + per-head RMSNorm + rotary + causal GQA attention + output
projection), running SPMD on 8 NeuronCores.

Sharding (rank-uniform, no divergent control flow):
  - 8 cores = 2 batch groups x 4 tensor-parallel ranks.
  - Core c: batch b = c // 4, rank r = c % 4.
  - QKV projection + attention are head-sharded: core r computes q heads
    4r..4r+3 and kv head r for ALL tokens of its batch.
  - One 8-rank AllToAll per head re-shards y from head-split to token-split
    (each rank stages its y token-quarters in both batch halves' chunk slots;
    receivers read only their own batch's half).  A2A moves 4x less wire
    data than the AllGather it replaces and runs ~13us vs ~42us.
  - The output projection for head h-1 is interleaved after head h's
    attention, so its matmuls fill the tensor-engine slack while the
    scalar engine (exp) paces attention.

Layout tricks:
  - Host pre-transposes x, wqkv, wo so the kernel's matmuls need no on-device
    transposes (except tiny 128x128 PE transposes for V).
  - Rope's even/odd pair swap is a 128x128 permutation matmul in bf16.
  - Scores are computed transposed [kv, q]; diagonal kv-blocks are column-
    trimmed (fully-masked q columns never computed), stored left-shifted so
    each diagonal pair still needs only ONE exp activation; the remaining
    128-wide causal staircase band uses a single shared triangular mask.
  - exp is fused into the PSUM->SBUF eviction on the ScalarEngine, batched
    two kv-blocks per activation to amortize the ~300-cycle overhead.
  - The softmax denominator is accumulated on the VectorEngine in bf16 and
    reduced across partitions with a ones-vector matmul; the reciprocal is
    broadcast back to 128 partitions with a K=1 matmul.  Both matmul
    outputs live in slices of the score-pair PSUM ring, freeing two PSUM
    banks for the interleaved output projection.
  - RMSNorm reduces to a per-token scalar via a ones-vector matmul over the
    squared tile; the 1/sqrt(head_dim) score scale folds into the q-side
    scalar.
  - All big matmuls run in bf16 with f32 PSUM accumulation.
  - x is staged through a 2-deep ring of per-token-chunk tiles in need
    order; wo panels prefetch during phase-1's idle DMA tail.
"""

import numpy as np
import ml_dtypes

import concourse.bass as bass
import concourse.mybir as mybir
import concourse.tile as tile
from concourse import bacc
from concourse.bass_utils import run_bass_kernel_spmd

BF16 = mybir.dt.bfloat16
F32 = mybir.dt.float32

DIM = 2048
NH = 16
NKV = 4
HD = 128
EPS = 1e-5
N_CORES = 8
RG8 = [[0, 1, 2, 3, 4, 5, 6, 7]]  # 8-rank group for the AllToAll

HL = NH // NKV  # q heads per core (= GQA group size) = 4
EW = HL * HD + 2 * HD  # wqkv column-slice width per core = 768
NDT = DIM // 128  # contraction tiles = 16
SWAP_MASK = [i ^ 1 for i in range(32)]  # rope pair swap, per 32-lane group


def build_graph(S, debug_taps=False):
    """Build + compile the SPMD graph for sequence length S. Returns nc."""
    TPT = S // 4       # tokens per core after the re-shard (output rows)
    TCW = S // 4       # token chunk width for phase 1 (moving dim <= 512)
    NTT = S // TCW     # number of token chunks = 4
    QC = 512           # attention q-chunk width
    KB = 128           # kv block size
    NQC = S // QC      # q chunks per head
    AVDEPTH = 3        # kv-block pairs the AV matmuls trail the score matmuls

    nc = bacc.Bacc("TRN2", target_bir_lowering=False, debug=False,
                   num_devices=N_CORES)

    # ---- DRAM I/O ----
    xT_d = nc.dram_tensor("xT", [DIM, S], BF16, kind="ExternalInput")
    w_d = nc.dram_tensor("wslice", [DIM, EW], BF16, kind="ExternalInput")
    wo_d = nc.dram_tensor("woT", [DIM, DIM], BF16, kind="ExternalInput")
    cos_d = nc.dram_tensor("cosF", [128, S], BF16, kind="ExternalInput")
    sin_d = nc.dram_tensor("sinF", [128, S], BF16, kind="ExternalInput")
    idn_d = nc.dram_tensor("ident", [128, 128], BF16, kind="ExternalInput")
    tri_d = nc.dram_tensor("tri", [KB, KB], BF16, kind="ExternalInput")
    qw_d = nc.dram_tensor("qw", [128, 1], F32, kind="ExternalInput")
    kw_d = nc.dram_tensor("kw", [128, 1], F32, kind="ExternalInput")
    out_d = nc.dram_tensor("out", [DIM, TPT], BF16, kind="ExternalOutput")
    if debug_taps:
        dbg_q = nc.dram_tensor("dbg_q", [128, HL * S], BF16,
                               kind="ExternalOutput")
        dbg_k = nc.dram_tensor("dbg_k", [128, S], BF16, kind="ExternalOutput")
        dbg_v = nc.dram_tensor("dbg_v", [128, S], BF16, kind="ExternalOutput")
        dbg_y = nc.dram_tensor("dbg_y", [128, HL * S], BF16,
                               kind="ExternalOutput")
        dbg_yf = nc.dram_tensor("dbg_yf", [128, HL * 4 * TPT], BF16,
                                kind="ExternalOutput")

    with tile.TileContext(nc) as tc:
        with tc.tile_pool(name="const", bufs=1) as cpool, \
             tc.tile_pool(name="big", bufs=1) as bigpool, \
             tc.tile_pool(name="wo", bufs=1) as wopool, \
             tc.tile_pool(name="dram", bufs=1, space="DRAM") as dpool:

            # constants (gpsimd queue: keep the sync queue free for weights)
            idn = cpool.tile([128, 128], BF16, tag="idn")
            nc.gpsimd.dma_start(idn[:], idn_d[:])
            tri = cpool.tile([KB, KB], BF16, tag="tri")
            nc.gpsimd.dma_start(tri[:], tri_d[:])
            qw = cpool.tile([128, 1], F32, tag="qw")
            nc.gpsimd.dma_start(qw[:], qw_d[:])
            kw = cpool.tile([128, 1], F32, tag="kw")
            nc.gpsimd.dma_start(kw[:], kw_d[:])
            ones = cpool.tile([128, 1], BF16, tag="ones")
            nc.vector.memset(ones[:], 1.0)
            onec = cpool.tile([1, 128], BF16, tag="onec")
            nc.vector.memset(onec[:], 1.0)
            b0 = cpool.tile([128, 1], F32, tag="b0")
            nc.vector.memset(b0[:], 0.0)
            bq = cpool.tile([1, 1], F32, tag="bq")
            nc.vector.memset(bq[:], float(HD * EPS))
            bk = cpool.tile([1, 1], F32, tag="bk")
            nc.vector.memset(bk[:], float(EPS))

            # long-lived activations
            qT = bigpool.tile([128, HL * S], BF16, tag="qT")
            kT = bigpool.tile([128, S], BF16, tag="kT")
            V = bigpool.tile([128, S], BF16, tag="V")   # [tok%128, blk*128+d]

            # output-projection weight panels; the DMAs are issued during
            # phase 1 so they ride its idle DMA tail
            wo_h = [wopool.tile([128, 4 * S], BF16, tag=f"wo{h}",
                                name=f"wo{h}")
                    for h in range(HL)]

            # tiny warm-up AllToAll: pays the collective firmware's cold
            # start in the shadow of phase-1 compute so the real per-head
            # re-shards fire promptly
            wu_in = dpool.tile([8 * 16, 16], BF16, tag="wuin", name="wu_in")
            wu_out = dpool.tile([8 * 16, 16], BF16, tag="wuout",
                                name="wu_out")
            nc.gpsimd.collective_compute(
                "AllToAll", mybir.AluOpType.bypass, replica_groups=RG8,
                ins=[wu_in.opt()], outs=[wu_out.opt()])

            # ---------------- Phase 1: QKV + norm + rope ----------------
            with tc.tile_pool(name="wq", bufs=1) as wpool, \
                 tc.tile_pool(name="x", bufs=3) as xpool, \
                 tc.tile_pool(name="cs", bufs=2) as cspool, \
                 tc.tile_pool(name="scr", bufs=2) as scr, \
                 tc.tile_pool(name="smol", bufs=2) as smol, \
                 tc.tile_pool(name="p1", bufs=5, space="PSUM") as p1, \
                 tc.tile_pool(name="pss", bufs=1, space="PSUM") as pss, \
                 tc.tile_pool(name="pvt", bufs=2, space="PSUM") as pvt:

                # full wqkv slice, staged once: [128, dt*EW + e]
                w_sb = wpool.tile([128, NDT * EW], BF16, tag="w")

                xr = [None] * NTT

                def issue_x(tt):
                    xr[tt] = xpool.tile([128, NDT * TCW], BF16, tag="x",
                                        name=f"x{tt}")
                    for dt in range(NDT):
                        # tt0 rides the scalar queue alone so it is never
                        # queued behind the 3MB of weights on sync
                        eng = (nc.scalar if (tt == 0 or dt % 2 == 0)
                               else nc.sync)
                        eng.dma_start(
                            xr[tt][:, dt * TCW:(dt + 1) * TCW],
                            xT_d[dt * 128:(dt + 1) * 128,
                                 tt * TCW:(tt + 1) * TCW])

                def process_qk(ps, et, tt, cos_t, sin_t):
                    is_q = et < HL
                    # sum of squares over head_dim via ones-vector matmul
                    sqv = smol.tile([128, TCW], BF16, tag="sq2", name="sqv")
                    nc.scalar.activation(
                        sqv[:], ps[:],
                        mybir.ActivationFunctionType.Square, bias=b0[:])
                    ss = pss.tile([1, TCW], F32, tag="ss", name="ss")
                    nc.tensor.matmul(ss[:], ones[:], sqv[:],
                                     start=True, stop=True)
                    sq = smol.tile([1, TCW], F32, tag="sqs", name="sq")
                    if is_q:
                        # 1/sqrt(ss + HD*eps) folds the 1/sqrt(HD) score scale
                        nc.scalar.activation(
                            sq[:], ss[:],
                            mybir.ActivationFunctionType.Sqrt,
                            bias=bq[:], scale=1.0)
                    else:
                        nc.scalar.activation(
                            sq[:], ss[:],
                            mybir.ActivationFunctionType.Sqrt,
                            bias=bk[:], scale=1.0 / HD)
                    inv = smol.tile([1, TCW], F32, tag="inv", name="inv")
                    nc.vector.reciprocal_approx_fast(inv[:], sq[:])
                    invb = scr.tile([128, TCW], F32, tag="invb", name="invb")
                    nc.gpsimd.partition_broadcast(invb[:], inv[:])
                    qf = scr.tile([128, TCW], BF16, tag="qf", name="qf")
                    nc.scalar.mul(qf[:], ps[:], (qw if is_q else kw)[:])
                    # rope: pair swap on the DVE lane shuffler, sinF signed
                    sw = scr.tile([128, TCW], BF16, tag="sw", name="sw")
                    nc.vector.stream_shuffle(sw[:], qf[:], SWAP_MASK)
                    t1 = scr.tile([128, TCW], F32, tag="t1", name="t1")
                    nc.vector.tensor_mul(t1[:], qf[:], cos_t[:])
                    t2 = scr.tile([128, TCW], F32, tag="t2", name="t2")
                    nc.vector.tensor_mul(t2[:], sw[:], sin_t[:])
                    nc.vector.tensor_add(t1[:], t1[:], t2[:])
                    dst = (qT[:, et * S + tt * TCW: et * S + tt * TCW + TCW]
                           if is_q else
                           kT[:, tt * TCW: tt * TCW + TCW])
                    nc.vector.tensor_mul(dst, t1[:], invb[:])

                def process_v(ps, tt):
                    vb = smol.tile([128, TCW], BF16, tag="vb", name="vb")
                    nc.scalar.copy(vb[:], ps[:])
                    for bb in range(TCW // 128):
                        tp = pvt.tile([128, 128], BF16, tag="tp", name="tp")
                        nc.tensor.transpose(
                            tp[:], vb[:, bb * 128:(bb + 1) * 128], idn[:])
                        blk = tt * (TCW // 128) + bb
                        nc.scalar.copy(V[:, blk * 128:(blk + 1) * 128], tp[:])

                # weights + first token chunk in need-order
                for dt in range(NDT):
                    nc.sync.dma_start(
                        w_sb[:, dt * EW:(dt + 1) * EW],
                        w_d[dt * 128:(dt + 1) * 128, :])
                issue_x(0)

                pend = []  # (psum, et, tt, cos_t, sin_t) awaiting processing

                def process_one():
                    pps, pet, ptt, pc, psn_ = pend.pop(0)
                    if pet < HL + 1:
                        process_qk(pps, pet, ptt, pc, psn_)
                    else:
                        process_v(pps, ptt)

                for tt in range(NTT):
                    cos_t = cspool.tile([128, TCW], BF16, tag="cos")
                    nc.sync.dma_start(cos_t[:], cos_d[:, tt * TCW:(tt + 1) * TCW])
                    sin_t = cspool.tile([128, TCW], BF16, tag="sin")
                    nc.sync.dma_start(sin_t[:], sin_d[:, tt * TCW:(tt + 1) * TCW])
                    if tt + 1 < NTT:
                        issue_x(tt + 1)
                    if tt == NTT - 1:
                        # wo prefetch rides phase-1's idle DMA tail
                        for h in range(HL):
                            for j in range(4):
                                et = 4 * j + h
                                nc.sync.dma_start(
                                    wo_h[h][:, j * S:(j + 1) * S],
                                    wo_d[et * 128:(et + 1) * 128, :])

                    # k and v first: attention's inputs finish earliest and
                    # the end-of-phase drain holds only late q heads
                    ets = [HL, HL + 1] + list(range(HL))
                    first_ei = 0
                    if tt == 0:
                        # dt-outer warm-up triple: the matmul stream paces
                        # the (w[dt], x[dt]) DMA trickle instead of stalling
                        # on the full 5MB prefix before the first group
                        first_ei = 3
                        trip = [p1.tile([128, TCW], F32, tag="ps",
                                        name=f"ps_w{ei}") for ei in range(3)]
                        for dt in range(NDT):
                            for ei in range(3):
                                et = ets[ei]
                                nc.tensor.matmul(
                                    trip[ei][:],
                                    w_sb[:, dt * EW + et * 128:
                                         dt * EW + (et + 1) * 128],
                                    xr[0][:, dt * TCW:(dt + 1) * TCW],
                                    start=(dt == 0), stop=(dt == NDT - 1),
                                )
                        for ei in range(3):
                            pend.append((trip[ei], ets[ei], 0, cos_t, sin_t))

                    for et in ets[first_ei:]:
                        ps = p1.tile([128, TCW], F32, tag="ps")
                        for dt in range(NDT):
                            nc.tensor.matmul(
                                ps[:],
                                w_sb[:, dt * EW + et * 128:dt * EW + (et + 1) * 128],
                                xr[tt][:, dt * TCW:(dt + 1) * TCW],
                                start=(dt == 0), stop=(dt == NDT - 1),
                            )
                        # process an older tile now: its cross-engine waits
                        # overlap this tile's matmul group
                        if pend:
                            process_one()
                        if tt == NTT - 1 and pend:
                            process_one()  # eager drain: shallow phase exit
                        pend.append((ps, et, tt, cos_t, sin_t))
                while pend:
                    process_one()

            if debug_taps:
                nc.sync.dma_start(dbg_q[:], qT[:])
                nc.sync.dma_start(dbg_k[:], kT[:])
                nc.sync.dma_start(dbg_v[:], V[:])

            # ------- Phase 2: causal attention + interleaved outproj -------
            with tc.tile_pool(name="part", bufs=1) as partpool, \
                 tc.tile_pool(name="yf", bufs=1) as yfpool, \
                 tc.tile_pool(name="yt", bufs=2) as ytpool, \
                 tc.tile_pool(name="acc", bufs=2) as accpool, \
                 tc.tile_pool(name="exp", bufs=8) as epool, \
                 tc.tile_pool(name="rs", bufs=2) as rspool, \
                 tc.tile_pool(name="ot", bufs=2) as otpool, \
                 tc.tile_pool(name="pa", bufs=2, space="PSUM") as pa, \
                 tc.tile_pool(name="py", bufs=2, space="PSUM") as py, \
                 tc.tile_pool(name="pd", bufs=2, space="PSUM") as pd:

                part = partpool.tile([128, NDT * TPT], F32, tag="part")
                yf_h = [yfpool.tile([128, 4 * TPT], BF16, tag=f"yf{h}",
                                    name=f"yf{h}")
                        for h in range(HL)]
                pid = nc.gpsimd.partition_id()
                # token-quarter base of this rank, on the PE register file
                # (it feeds a matmul moving-operand offset)
                pid_pe = nc.tensor.partition_id()
                roff = nc.s_assert_within((pid_pe % 4) * TPT, 0, S - TPT,
                                          skip_runtime_assert=True)

                def op_tile():
                    # outproj groups ride the score-pair PSUM ring
                    # (attention is quiescent while an outproj block runs)
                    ps_ot = pa.tile([128, 2 * QC], F32, tag="s",
                                    name="ps_ot")
                    return ps_ot[:, 0:TPT]

                def op01(ot):
                    # heads 0+1 accumulate in one PSUM group: 1 evict per ot
                    ps_o = op_tile()
                    for g in (0, 1):
                        for p in range(4):
                            nc.tensor.matmul(
                                ps_o,
                                wo_h[g][:, p * S + ot * 128: p * S + ot * 128 + 128],
                                yf_h[g][:, p * TPT:(p + 1) * TPT],
                                start=(g == 0 and p == 0),
                                stop=(g == 1 and p == 3))
                    # evict on ScalarE: DVE is the busier engine mid-window
                    nc.scalar.copy(part[:, ot * TPT:(ot + 1) * TPT], ps_o)

                def op2_3self(ot, yT3):
                    # head 2 (re-shard done long ago) + head 3's own-rank
                    # quarter straight out of local yT: runs BEFORE the last
                    # AllToAll lands
                    ps_o = op_tile()
                    for p in range(4):
                        nc.tensor.matmul(
                            ps_o,
                            wo_h[2][:, p * S + ot * 128: p * S + ot * 128 + 128],
                            yf_h[2][:, p * TPT:(p + 1) * TPT],
                            start=(p == 0), stop=False)
                    nc.tensor.matmul(
                        ps_o,
                        wo_h[3][:, 0 * S + ot * 128: 0 * S + ot * 128 + 128],
                        yT3[:, bass.ds(roff, TPT)],
                        start=False, stop=True)
                    psl = part[:, ot * TPT:(ot + 1) * TPT]
                    nc.vector.tensor_add(psl, psl, ps_o)

                def op3rest(ot):
                    # head 3's three remote quarters: the only work gated on
                    # the final AllToAll
                    ps_o = op_tile()
                    for p in (1, 2, 3):
                        nc.tensor.matmul(
                            ps_o,
                            wo_h[3][:, p * S + ot * 128: p * S + ot * 128 + 128],
                            yf_h[3][:, p * TPT:(p + 1) * TPT],
                            start=(p == 1), stop=(p == 3))
                    ott = otpool.tile([128, TPT], BF16, tag="ot", name="ott")
                    nc.vector.tensor_add(
                        ott[:], ps_o, part[:, ot * TPT:(ot + 1) * TPT])
                    nc.sync.dma_start(out_d[ot * 128:(ot + 1) * 128, :],
                                      ott[:])

                for h in range(HL):
                    yT = ytpool.tile([128, S], BF16, tag="yT", name="yT")
                    in_b = dpool.tile([8 * 128, TPT], BF16, tag=f"a2i{h}",
                                      name=f"a2ain{h}")
                    for qc in range(NQC):
                        nblk = 4 * (qc + 1)
                        nfull = 4 * qc
                        ps_y = py.tile([128, QC], F32, tag="y", name="ps_y")
                        acc = accpool.tile([128, QC], BF16, tag="acc",
                                           name="acc")
                        qsl = qT[:, h * S + qc * QC: h * S + (qc + 1) * QC]

                        pend_av = []  # (ex2, ga, diag) awaiting AV matmuls

                        def emit_av(ex2, ga, diag):
                            if not diag:
                                for g, off in ((ga, 0), (ga + 1, QC)):
                                    nc.tensor.matmul(
                                        ps_y[:],
                                        V[:, g * 128:(g + 1) * 128],
                                        ex2[:, off: off + QC],
                                        start=(g == 0), stop=(g == nblk - 1))
                            else:
                                ta = ga - nfull
                                w0 = QC - ta * KB
                                w1 = QC - (ta + 1) * KB
                                nc.tensor.matmul(
                                    ps_y[:, ta * KB:QC],
                                    V[:, ga * 128:(ga + 1) * 128],
                                    ex2[:, 0:w0],
                                    start=(ga == 0), stop=False)
                                nc.tensor.matmul(
                                    ps_y[:, (ta + 1) * KB:QC],
                                    V[:, (ga + 1) * 128:(ga + 2) * 128],
                                    ex2[:, w0:w0 + w1],
                                    start=False, stop=(ga + 1 == nblk - 1))

                        # full (unmasked) kv-block pairs
                        for p in range(nfull // 2):
                            ga = 2 * p
                            pa2 = pa.tile([128, 2 * QC], F32, tag="s",
                                          name="pa2")
                            nc.tensor.matmul(
                                pa2[:, 0:QC],
                                kT[:, ga * KB:(ga + 1) * KB],
                                qsl, start=True, stop=True)
                            nc.tensor.matmul(
                                pa2[:, QC:2 * QC],
                                kT[:, (ga + 1) * KB:(ga + 2) * KB],
                                qsl, start=True, stop=True)
                            ex2 = epool.tile([128, 2 * QC], BF16, tag="e",
                                             name="ex2")
                            nc.scalar.activation(
                                ex2[:], pa2[:],
                                mybir.ActivationFunctionType.Exp, bias=b0[:])
                            # denominator accumulation on DVE (bf16)
                            if p == 0:
                                nc.vector.tensor_add(
                                    acc[:], ex2[:, 0:QC], ex2[:, QC:2 * QC])
                            else:
                                ap = epool.tile([128, QC], BF16, tag="ap",
                                                name="accp")
                                nc.vector.tensor_add(
                                    ap[:], ex2[:, 0:QC], ex2[:, QC:2 * QC])
                                nc.vector.tensor_add(acc[:], acc[:], ap[:])
                            pend_av.append((ex2, ga, False))
                            if len(pend_av) > AVDEPTH:
                                emit_av(*pend_av.pop(0))

                        # diagonal pairs: column-trimmed, left-shifted scores
                        for dp_i in range(2):
                            ta0 = 2 * dp_i
                            ta1 = ta0 + 1
                            ga = nfull + ta0
                            w0 = QC - ta0 * KB
                            w1 = QC - ta1 * KB
                            pa2 = pa.tile([128, 2 * QC], F32, tag="s",
                                          name="pa2d")
                            nc.tensor.matmul(
                                pa2[:, 0:w0],
                                kT[:, ga * KB:(ga + 1) * KB],
                                qsl[:, ta0 * KB:QC], start=True, stop=True)
                            nc.tensor.matmul(
                                pa2[:, w0:w0 + w1],
                                kT[:, (ga + 1) * KB:(ga + 2) * KB],
                                qsl[:, ta1 * KB:QC], start=True, stop=True)
                            ex2 = epool.tile([128, 2 * QC], BF16, tag="e",
                                             name="ex2d")
                            nc.scalar.activation(
                                ex2[:, 0:w0 + w1], pa2[:, 0:w0 + w1],
                                mybir.ActivationFunctionType.Exp, bias=b0[:])
                            # causal staircase bands (one shared triangle)
                            nc.vector.tensor_mul(
                                ex2[:, 0:KB], ex2[:, 0:KB], tri[:])
                            nc.vector.tensor_mul(
                                ex2[:, w0:w0 + KB], ex2[:, w0:w0 + KB],
                                tri[:])
                            # denominator accumulation (aligned slices)
                            if qc == 0 and dp_i == 0:
                                nc.vector.tensor_copy(acc[:], ex2[:, 0:w0])
                            else:
                                nc.vector.tensor_add(
                                    acc[:, ta0 * KB:QC],
                                    acc[:, ta0 * KB:QC], ex2[:, 0:w0])
                            nc.vector.tensor_add(
                                acc[:, ta1 * KB:QC],
                                acc[:, ta1 * KB:QC], ex2[:, w0:w0 + w1])
                            pend_av.append((ex2, ga, True))
                            if len(pend_av) > AVDEPTH:
                                emit_av(*pend_av.pop(0))
                        for args in pend_av:
                            emit_av(*args)

                        # denominator: ones-matmul over the bf16 accumulator;
                        # reciprocal broadcast back to 128 partitions via a
                        # K=1 matmul.  den has its own bank; the broadcast
                        # rides the ps_y ring (its slot frees exactly when
                        # the previous chunk's normalize completes).
                        den = pd.tile([1, QC], F32, tag="den", name="den")
                        nc.tensor.matmul(den[:], ones[:], acc[:],
                                         start=True, stop=True)
                        rec1 = rspool.tile([1, QC], F32, tag="rc1",
                                           name="rec1")
                        nc.vector.reciprocal_approx_fast(rec1[:], den[:])
                        rc16 = rspool.tile([1, QC], BF16, tag="rc6",
                                           name="rc16")
                        nc.vector.tensor_copy(rc16[:], rec1[:])
                        rect = py.tile([128, QC], F32, tag="y", name="rect")
                        rec = rect[:, 0:QC]
                        nc.tensor.matmul(rec, onec[:], rc16[:],
                                         start=True, stop=True)
                        rsb = rspool.tile([128, QC], F32, tag="rsb",
                                          name="rsb")
                        nc.vector.tensor_copy(rsb[:], rec)
                        nc.vector.tensor_mul(
                            yT[:, qc * QC:(qc + 1) * QC], ps_y[:], rsb[:])
                        # stage this token-quarter into both batch halves of
                        # the AllToAll input
                        nc.sync.dma_start(
                            in_b[qc * 128:(qc + 1) * 128, :],
                            yT[:, qc * QC:(qc + 1) * QC])
                        nc.sync.dma_start(
                            in_b[(4 + qc) * 128:(5 + qc) * 128, :],
                            yT[:, qc * QC:(qc + 1) * QC])

                    if debug_taps:
                        nc.sync.dma_start(dbg_y[:, h * S:(h + 1) * S], yT[:])

                    # per-head 8-rank AllToAll re-shard (head- -> token-split)
                    out_b = dpool.tile([8 * 128, TPT], BF16, tag=f"a2o{h}",
                                       name=f"a2aout{h}")
                    nc.gpsimd.collective_compute(
                        "AllToAll", mybir.AluOpType.bypass,
                        replica_groups=RG8,
                        ins=[in_b.opt()], outs=[out_b.opt()])
                    # readback queued behind the A2A on the gpsimd queue:
                    # fires the moment the collective completes.  Chunks are
                    # permuted so panel p holds sender (rk+p)%4 — matching
                    # the host-side wo panel permutation and making panel 0
                    # always this rank's own head.
                    for p in range(4):
                        row = nc.s_assert_within(
                            (pid - pid % 4 + (pid % 4 + p) % 4) * 128,
                            0, 896, skip_runtime_assert=True)
                        nc.gpsimd.dma_start(
                            yf_h[h][:, p * TPT:(p + 1) * TPT],
                            out_b[bass.ds(row, 128), :])

                    if debug_taps:
                        nc.sync.dma_start(
                            dbg_yf[:, h * 4 * TPT:(h + 1) * 4 * TPT],
                            yf_h[h][:])

                    # output projections interleave so the re-shard of head
                    # g is always complete (with slack for rank skew) before
                    # the in-order PE queue reaches it:
                    #   attn0 attn1 attn2 [op0+op1] attn3 [op2+op3self] [op3rest]
                    if h == 2:
                        for ot in range(NDT):
                            op01(ot)
                    if h == 3:
                        yT3 = yT
                for ot in range(NDT):
                    op2_3self(ot, yT3)
                for ot in range(NDT):
                    op3rest(ot)

    nc.compile()
    return nc


def make_in_maps(x, freqs_cis, wqkv, wo, q_norm_w, k_norm_w, S):
    """Host-side sharding / layout prep. Returns list of 8 input dicts."""
    bf = ml_dtypes.bfloat16
    KB = 128

    # rope tables: [128, S]; row 2i & 2i+1 carry cos[t, i]; sin signed
    cos = np.asarray(freqs_cis[:S, :, 0], np.float32)   # [S, 64]
    sin = np.asarray(freqs_cis[:S, :, 1], np.float32)
    cosF = np.ascontiguousarray(np.repeat(cos.T, 2, axis=0)).astype(bf)
    sinF = np.repeat(sin.T, 2, axis=0).astype(np.float32)
    sinF[0::2] *= -1.0
    sinF = np.ascontiguousarray(sinF).astype(bf)

    ident = np.eye(128, dtype=bf)

    # shared causal staircase triangle: allowed iff kv-row r <= stored col p
    r = np.arange(KB)[:, None]
    p = np.arange(KB)[None, :]
    tri = (r <= p).astype(np.float32).astype(bf)

    qwv = np.asarray(q_norm_w, np.float32).reshape(128, 1)
    kwv = np.asarray(k_norm_w, np.float32).reshape(128, 1)

    woT = np.ascontiguousarray(np.asarray(wo, np.float32).T).astype(bf)
    # per-core panel permutation: row-block (4p+g) holds the wo columns of
    # head 4*((rk+p)%4)+g, so in-kernel panel p is sender (rk+p)%4 and
    # panel 0 is always the core's own head
    woT_core = []
    for rk in range(4):
        wc = np.empty_like(woT)
        for p in range(4):
            for g in range(4):
                src = 4 * ((rk + p) % 4) + g
                dst = 4 * p + g
                wc[dst * 128:(dst + 1) * 128] = woT[src * 128:(src + 1) * 128]
        woT_core.append(np.ascontiguousarray(wc))

    xTb = []
    for b in range(2):
        xTb.append(np.ascontiguousarray(np.asarray(x[b], np.float32).T)
                   .astype(bf))

    wq = np.asarray(wqkv, np.float32)
    q_sz = NH * HD
    in_maps = []
    for c_id in range(N_CORES):
        b, rk = c_id // 4, c_id % 4
        rows = np.concatenate([
            wq[rk * HL * HD:(rk + 1) * HL * HD],          # 4 q heads
            wq[q_sz + rk * HD: q_sz + (rk + 1) * HD],     # k head
            wq[q_sz + NKV * HD + rk * HD:
               q_sz + NKV * HD + (rk + 1) * HD],          # v head
        ], axis=0)                                        # [768, 2048]
        wslice = np.ascontiguousarray(rows.T).astype(bf)  # [2048, 768]
        in_maps.append({
            "xT": xTb[b], "wslice": wslice, "woT": woT_core[rk],
            "cosF": cosF, "sinF": sinF,
            "ident": ident, "tri": tri,
            "qw": qwv, "kw": kwv,
        })
    return in_maps


_NC_CACHE = {}


def kernel(x, freqs_cis, mask, wqkv, wo, q_norm_w, k_norm_w):
    x = np.asarray(x)
    S = x.shape[1]
    if S not in _NC_CACHE:
        _NC_CACHE[S] = build_graph(S)
    nc = _NC_CACHE[S]
    in_maps = make_in_maps(x, freqs_cis, wqkv, wo, q_norm_w, k_norm_w, S)
    res = run_bass_kernel_spmd(nc, in_maps, core_ids=list(range(N_CORES)))
    TPT = S // 4
    out = np.empty((2, S, DIM), np.float32)
    for c_id in range(N_CORES):
        b, rk = c_id // 4, c_id % 4
        out[b, rk * TPT:(rk + 1) * TPT, :] = res.results[c_id]["out"].T.astype(np.float32)
    return out


# revision 41
# speedup vs baseline: 1.0103x; 1.0103x over previous
"""Distributed Trainium2 kernel for nn_Attention (dense transformer block:
fused QKV projection + per-head RMSNorm + rotary + causal GQA attention + output
projection), running SPMD on 8 NeuronCores.

Sharding (rank-uniform, no divergent control flow):
  - 8 cores = 2 batch groups x 4 tensor-parallel ranks.
  - Core c: batch b = c // 4, rank r = c % 4.
  - QKV projection + attention are head-sharded: core r computes q heads
    4r..4r+3 and kv head r for ALL tokens of its batch.
  - One 8-rank AllToAll per head re-shards y from head-split to token-split
    (each rank stages its y token-quarters in both batch halves' chunk slots;
    receivers read only their own batch's half).  A2A moves 4x less wire
    data than the AllGather it replaces and runs ~13us vs ~42us.
  - The output projection for head h-1 is interleaved after head h's
    attention, so its matmuls fill the tensor-engine slack while the
    scalar engine (exp) paces attention.

Layout tricks:
  - Host pre-transposes x, wqkv, wo so the kernel's matmuls need no on-device
    transposes (except tiny 128x128 PE transposes for V).
  - Rope's even/odd pair swap is a 128x128 permutation matmul in bf16.
  - Scores are computed transposed [kv, q]; diagonal kv-blocks are column-
    trimmed (fully-masked q columns never computed), stored left-shifted so
    each diagonal pair still needs only ONE exp activation; the remaining
    128-wide causal staircase band uses a single shared triangular mask.
  - exp is fused into the PSUM->SBUF eviction on the ScalarEngine, batched
    two kv-blocks per activation to amortize the ~300-cycle overhead.
  - The softmax denominator is accumulated on the VectorEngine in bf16 and
    reduced across partitions with a ones-vector matmul; the reciprocal is
    broadcast back to 128 partitions with a K=1 matmul.  Both matmul
    outputs live in slices of the score-pair PSUM ring, freeing two PSUM
    banks for the interleaved output projection.
  - RMSNorm reduces to a per-token scalar via a ones-vector matmul over the
    squared tile; the 1/sqrt(head_dim) score scale folds into the q-side
    scalar.
  - All big matmuls run in bf16 with f32 PSUM accumulation.
  - x is staged through a 2-deep ring of per-token-chunk tiles in need
    order; wo panels prefetch during phase-1's idle DMA tail.
"""

import numpy as np
import ml_dtypes

import concourse.bass as bass
import concourse.mybir as mybir
import concourse.tile as tile
from concourse import bacc
from concourse.bass_utils import run_bass_kernel_spmd

BF16 = mybir.dt.bfloat16
F32 = mybir.dt.float32

DIM = 2048
NH = 16
NKV = 4
HD = 128
EPS = 1e-5
N_CORES = 8
RG8 = [[0, 1, 2, 3, 4, 5, 6, 7]]  # 8-rank group for the AllToAll

HL = NH // NKV  # q heads per core (= GQA group size) = 4
EW = HL * HD + 2 * HD  # wqkv column-slice width per core = 768
NDT = DIM // 128  # contraction tiles = 16
SWAP_MASK = [i ^ 1 for i in range(32)]  # rope pair swap, per 32-lane group


def build_graph(S, debug_taps=False):
    """Build + compile the SPMD graph for sequence length S. Returns nc."""
    TPT = S // 4       # tokens per core after the re-shard (output rows)
    TCW = S // 4       # token chunk width for phase 1 (moving dim <= 512)
    NTT = S // TCW     # number of token chunks = 4
    QC = 512           # attention q-chunk width
    KB = 128           # kv block size
    NQC = S // QC      # q chunks per head
    AVDEPTH = 3        # kv-block pairs the AV matmuls trail the score matmuls

    nc = bacc.Bacc("TRN2", target_bir_lowering=False, debug=False,
                   num_devices=N_CORES)

    # ---- DRAM I/O ----
    xT_d = nc.dram_tensor("xT", [DIM, S], BF16, kind="ExternalInput")
    w_d = nc.dram_tensor("wslice", [DIM, EW], BF16, kind="ExternalInput")
    wo_d = nc.dram_tensor("woT", [DIM, DIM], BF16, kind="ExternalInput")
    cos_d = nc.dram_tensor("cosF", [128, S], BF16, kind="ExternalInput")
    sin_d = nc.dram_tensor("sinF", [128, S], BF16, kind="ExternalInput")
    idn_d = nc.dram_tensor("ident", [128, 128], BF16, kind="ExternalInput")
    tri_d = nc.dram_tensor("tri", [KB, KB], BF16, kind="ExternalInput")
    qw_d = nc.dram_tensor("qw", [128, 1], F32, kind="ExternalInput")
    kw_d = nc.dram_tensor("kw", [128, 1], F32, kind="ExternalInput")
    out_d = nc.dram_tensor("out", [DIM, TPT], BF16, kind="ExternalOutput")
    if debug_taps:
        dbg_q = nc.dram_tensor("dbg_q", [128, HL * S], BF16,
                               kind="ExternalOutput")
        dbg_k = nc.dram_tensor("dbg_k", [128, S], BF16, kind="ExternalOutput")
        dbg_v = nc.dram_tensor("dbg_v", [128, S], BF16, kind="ExternalOutput")
        dbg_y = nc.dram_tensor("dbg_y", [128, HL * S], BF16,
                               kind="ExternalOutput")
        dbg_yf = nc.dram_tensor("dbg_yf", [128, HL * 4 * TPT], BF16,
                                kind="ExternalOutput")

    with tile.TileContext(nc) as tc:
        with tc.tile_pool(name="const", bufs=1) as cpool, \
             tc.tile_pool(name="big", bufs=1) as bigpool, \
             tc.tile_pool(name="wo", bufs=1) as wopool, \
             tc.tile_pool(name="exp", bufs=6) as epool, \
             tc.tile_pool(name="acc", bufs=2) as accpool, \
             tc.tile_pool(name="yt", bufs=2) as ytpool, \
             tc.tile_pool(name="dram", bufs=1, space="DRAM") as dpool:

            # constants (gpsimd queue: keep the sync queue free for weights)
            idn = cpool.tile([128, 128], BF16, tag="idn")
            nc.gpsimd.dma_start(idn[:], idn_d[:])
            tri = cpool.tile([KB, KB], BF16, tag="tri")
            nc.gpsimd.dma_start(tri[:], tri_d[:])
            qw = cpool.tile([128, 1], F32, tag="qw")
            nc.gpsimd.dma_start(qw[:], qw_d[:])
            kw = cpool.tile([128, 1], F32, tag="kw")
            nc.gpsimd.dma_start(kw[:], kw_d[:])
            ones = cpool.tile([128, 1], BF16, tag="ones")
            nc.vector.memset(ones[:], 1.0)
            onec = cpool.tile([1, 128], BF16, tag="onec")
            nc.vector.memset(onec[:], 1.0)
            b0 = cpool.tile([128, 1], F32, tag="b0")
            nc.vector.memset(b0[:], 0.0)
            bq = cpool.tile([1, 1], F32, tag="bq")
            nc.vector.memset(bq[:], float(HD * EPS))
            bk = cpool.tile([1, 1], F32, tag="bk")
            nc.vector.memset(bk[:], float(EPS))

            # long-lived activations
            qT = bigpool.tile([128, HL * S], BF16, tag="qT")
            kT = bigpool.tile([128, S], BF16, tag="kT")
            V = bigpool.tile([128, S], BF16, tag="V")   # [tok%128, blk*128+d]

            # output-projection weight panels; the DMAs are issued during
            # phase 1 so they ride its idle DMA tail
            wo_h = [wopool.tile([128, 4 * S], BF16, tag=f"wo{h}",
                                name=f"wo{h}")
                    for h in range(HL)]

            # tiny warm-up AllToAll: pays the collective firmware's cold
            # start in the shadow of phase-1 compute so the real per-head
            # re-shards fire promptly
            wu_in = dpool.tile([8 * 16, 16], BF16, tag="wuin", name="wu_in")
            wu_out = dpool.tile([8 * 16, 16], BF16, tag="wuout",
                                name="wu_out")
            nc.gpsimd.collective_compute(
                "AllToAll", mybir.AluOpType.bypass, replica_groups=RG8,
                ins=[wu_in.opt()], outs=[wu_out.opt()])

            # ---------------- Phase 1: QKV + norm + rope ----------------
            with tc.tile_pool(name="wq", bufs=1) as wpool, \
                 tc.tile_pool(name="x", bufs=2) as xpool, \
                 tc.tile_pool(name="cs", bufs=2) as cspool, \
                 tc.tile_pool(name="scr", bufs=2) as scr, \
                 tc.tile_pool(name="smol", bufs=2) as smol, \
                 tc.tile_pool(name="p1", bufs=5, space="PSUM") as p1, \
                 tc.tile_pool(name="pss", bufs=1, space="PSUM") as pss, \
                 tc.tile_pool(name="pvt", bufs=2, space="PSUM") as pvt:

                # full wqkv slice, staged once: [128, dt*EW + e]
                w_sb = wpool.tile([128, NDT * EW], BF16, tag="w")

                xr = [None] * NTT

                def issue_x(tt):
                    xr[tt] = xpool.tile([128, NDT * TCW], BF16, tag="x",
                                        name=f"x{tt}")
                    for dt in range(NDT):
                        # tt0 rides the scalar queue alone so it is never
                        # queued behind the 3MB of weights on sync
                        eng = (nc.scalar if (tt == 0 or dt % 2 == 0)
                               else nc.sync)
                        eng.dma_start(
                            xr[tt][:, dt * TCW:(dt + 1) * TCW],
                            xT_d[dt * 128:(dt + 1) * 128,
                                 tt * TCW:(tt + 1) * TCW])

                def process_qk(ps, et, tt, cos_t, sin_t):
                    is_q = et < HL
                    # sum of squares over head_dim via ones-vector matmul
                    sqv = smol.tile([128, TCW], BF16, tag="sq2", name="sqv")
                    nc.scalar.activation(
                        sqv[:], ps[:],
                        mybir.ActivationFunctionType.Square, bias=b0[:])
                    ss = pss.tile([1, TCW], F32, tag="ss", name="ss")
                    nc.tensor.matmul(ss[:], ones[:], sqv[:],
                                     start=True, stop=True)
                    # rsqrt as exp(-0.5*ln(.)): keeps the WHOLE kernel inside
                    # the natural_log_exp activation-table set (no mid-kernel
                    # ~2.7us table reloads) and needs no DVE reciprocal
                    sq = smol.tile([1, TCW], F32, tag="sqs", name="sq")
                    if is_q:
                        # ln(ss + HD*eps): folds the 1/sqrt(HD) score scale
                        nc.scalar.activation(
                            sq[:], ss[:],
                            mybir.ActivationFunctionType.Ln,
                            bias=bq[:], scale=1.0)
                    else:
                        nc.scalar.activation(
                            sq[:], ss[:],
                            mybir.ActivationFunctionType.Ln,
                            bias=bk[:], scale=1.0 / HD)
                    inv = smol.tile([1, TCW], F32, tag="inv", name="inv")
                    nc.scalar.activation(
                        inv[:], sq[:],
                        mybir.ActivationFunctionType.Exp, scale=-0.5)
                    invb = scr.tile([128, TCW], F32, tag="invb", name="invb")
                    nc.gpsimd.partition_broadcast(invb[:], inv[:])
                    qf = scr.tile([128, TCW], BF16, tag="qf", name="qf")
                    nc.scalar.mul(qf[:], ps[:], (qw if is_q else kw)[:])
                    # rope: pair swap on the DVE lane shuffler, sinF signed
                    sw = scr.tile([128, TCW], BF16, tag="sw", name="sw")
                    nc.vector.stream_shuffle(sw[:], qf[:], SWAP_MASK)
                    t1 = scr.tile([128, TCW], F32, tag="t1", name="t1")
                    nc.vector.tensor_mul(t1[:], qf[:], cos_t[:])
                    t2 = scr.tile([128, TCW], F32, tag="t2", name="t2")
                    nc.vector.tensor_mul(t2[:], sw[:], sin_t[:])
                    nc.vector.tensor_add(t1[:], t1[:], t2[:])
                    dst = (qT[:, et * S + tt * TCW: et * S + tt * TCW + TCW]
                           if is_q else
                           kT[:, tt * TCW: tt * TCW + TCW])
                    nc.vector.tensor_mul(dst, t1[:], invb[:])

                def process_v(ps, tt):
                    vb = smol.tile([128, TCW], BF16, tag="vb", name="vb")
                    nc.scalar.copy(vb[:], ps[:])
                    for bb in range(TCW // 128):
                        tp = pvt.tile([128, 128], BF16, tag="tp", name="tp")
                        nc.tensor.transpose(
                            tp[:], vb[:, bb * 128:(bb + 1) * 128], idn[:])
                        blk = tt * (TCW // 128) + bb
                        nc.scalar.copy(V[:, blk * 128:(blk + 1) * 128], tp[:])

                # weights + first token chunk in need-order
                for dt in range(NDT):
                    nc.sync.dma_start(
                        w_sb[:, dt * EW:(dt + 1) * EW],
                        w_d[dt * 128:(dt + 1) * 128, :])
                issue_x(0)

                pend = []  # (psum, et, tt, cos_t, sin_t) awaiting processing

                def process_one():
                    pps, pet, ptt, pc, psn_ = pend.pop(0)
                    if pet < HL + 1:
                        process_qk(pps, pet, ptt, pc, psn_)
                    else:
                        process_v(pps, ptt)

                for tt in range(NTT):
                    cos_t = cspool.tile([128, TCW], BF16, tag="cos")
                    nc.sync.dma_start(cos_t[:], cos_d[:, tt * TCW:(tt + 1) * TCW])
                    sin_t = cspool.tile([128, TCW], BF16, tag="sin")
                    nc.sync.dma_start(sin_t[:], sin_d[:, tt * TCW:(tt + 1) * TCW])
                    if tt + 1 < NTT:
                        issue_x(tt + 1)
                    if tt == NTT - 1:
                        # wo prefetch rides phase-1's idle DMA tail
                        for h in range(HL):
                            for j in range(4):
                                et = 4 * j + h
                                nc.sync.dma_start(
                                    wo_h[h][:, j * S:(j + 1) * S],
                                    wo_d[et * 128:(et + 1) * 128, :])

                    # k and v first: attention's inputs finish earliest and
                    # the end-of-phase drain holds only late q heads
                    ets = [HL, HL + 1] + list(range(HL))
                    first_ei = 0
                    if tt == 0:
                        # dt-outer warm-up triple: the matmul stream paces
                        # the (w[dt], x[dt]) DMA trickle instead of stalling
                        # on the full 5MB prefix before the first group
                        first_ei = 3
                        trip = [p1.tile([128, TCW], F32, tag="ps",
                                        name=f"ps_w{ei}") for ei in range(3)]
                        for dt in range(NDT):
                            for ei in range(3):
                                et = ets[ei]
                                nc.tensor.matmul(
                                    trip[ei][:],
                                    w_sb[:, dt * EW + et * 128:
                                         dt * EW + (et + 1) * 128],
                                    xr[0][:, dt * TCW:(dt + 1) * TCW],
                                    start=(dt == 0), stop=(dt == NDT - 1),
                                )
                        for ei in range(3):
                            pend.append((trip[ei], ets[ei], 0, cos_t, sin_t))

                    for et in ets[first_ei:]:
                        ps = p1.tile([128, TCW], F32, tag="ps")
                        for dt in range(NDT):
                            nc.tensor.matmul(
                                ps[:],
                                w_sb[:, dt * EW + et * 128:dt * EW + (et + 1) * 128],
                                xr[tt][:, dt * TCW:(dt + 1) * TCW],
                                start=(dt == 0), stop=(dt == NDT - 1),
                            )
                        # process an older tile now: its cross-engine waits
                        # overlap this tile's matmul group
                        if pend:
                            process_one()
                        if tt == NTT - 1 and pend:
                            process_one()  # eager drain: shallow phase exit
                        pend.append((ps, et, tt, cos_t, sin_t))
                while pend:
                    process_one()

            if debug_taps:
                nc.sync.dma_start(dbg_q[:], qT[:])
                nc.sync.dma_start(dbg_k[:], kT[:])
                nc.sync.dma_start(dbg_v[:], V[:])

            # ------- Phase 2: causal attention, then output projection -----
            with tc.tile_pool(name="part", bufs=1) as partpool, \
                 tc.tile_pool(name="yf", bufs=1) as yfpool, \
                 tc.tile_pool(name="rs", bufs=2) as rspool, \
                 tc.tile_pool(name="ot", bufs=2) as otpool:

                part = partpool.tile([128, NDT * TPT], F32, tag="part")
                yf_h = [yfpool.tile([128, 4 * TPT], BF16, tag=f"yf{h}",
                                    name=f"yf{h}")
                        for h in range(HL)]
                pid = nc.gpsimd.partition_id()
                # token-quarter base of this rank, on the PE register file
                # (it feeds a matmul moving-operand offset)
                pid_pe = nc.tensor.partition_id()
                roff = nc.s_assert_within((pid_pe % 4) * TPT, 0, S - TPT,
                                          skip_runtime_assert=True)
                op_pool = [None]

                def op_tile():
                    ps_o = op_pool[0].tile([128, TPT], F32, tag="o",
                                           name="ps_o")
                    return ps_o[:]

                def op01(ot):
                    # heads 0+1 accumulate in one PSUM group: 1 evict per ot
                    ps_o = op_tile()
                    for g in (0, 1):
                        for p in range(4):
                            nc.tensor.matmul(
                                ps_o,
                                wo_h[g][:, p * S + ot * 128: p * S + ot * 128 + 128],
                                yf_h[g][:, p * TPT:(p + 1) * TPT],
                                start=(g == 0 and p == 0),
                                stop=(g == 1 and p == 3))
                    # evict on ScalarE: DVE is the busier engine mid-window
                    nc.scalar.copy(part[:, ot * TPT:(ot + 1) * TPT], ps_o)

                def op2_3self(ot, yT3):
                    # head 2 (re-shard done long ago) + head 3's own-rank
                    # quarter straight out of local yT: runs BEFORE the last
                    # AllToAll lands
                    ps_o = op_tile()
                    for p in range(4):
                        nc.tensor.matmul(
                            ps_o,
                            wo_h[2][:, p * S + ot * 128: p * S + ot * 128 + 128],
                            yf_h[2][:, p * TPT:(p + 1) * TPT],
                            start=(p == 0), stop=False)
                    nc.tensor.matmul(
                        ps_o,
                        wo_h[3][:, 0 * S + ot * 128: 0 * S + ot * 128 + 128],
                        yT3[:, bass.ds(roff, TPT)],
                        start=False, stop=True)
                    psl = part[:, ot * TPT:(ot + 1) * TPT]
                    nc.vector.tensor_add(psl, psl, ps_o)

                def op3rest(ot):
                    # head 3's three remote quarters: the only work gated on
                    # the final AllToAll
                    ps_o = op_tile()
                    for p in (1, 2, 3):
                        nc.tensor.matmul(
                            ps_o,
                            wo_h[3][:, p * S + ot * 128: p * S + ot * 128 + 128],
                            yf_h[3][:, p * TPT:(p + 1) * TPT],
                            start=(p == 1), stop=(p == 3))
                    ott = otpool.tile([128, TPT], BF16, tag="ot", name="ott")
                    nc.vector.tensor_add(
                        ott[:], ps_o, part[:, ot * TPT:(ot + 1) * TPT])
                    nc.sync.dma_start(out_d[ot * 128:(ot + 1) * 128, :],
                                      ott[:])

                attn_psum = tc.tile_pool(name="pa", bufs=2, space="PSUM")
                pa = attn_psum.__enter__()
                py_cm = tc.tile_pool(name="py", bufs=2, space="PSUM")
                py = py_cm.__enter__()
                pd_cm = tc.tile_pool(name="pd", bufs=2, space="PSUM")
                pd = pd_cm.__enter__()

                for h in range(HL):
                    yT = ytpool.tile([128, S], BF16, tag="yT", name="yT")
                    in_b = dpool.tile([8 * 128, TPT], BF16, tag=f"a2i{h}",
                                      name=f"a2ain{h}")
                    for qc in range(NQC):
                        nblk = 4 * (qc + 1)
                        nfull = 4 * qc
                        ps_y = py.tile([128, QC], F32, tag="y", name="ps_y")
                        acc = accpool.tile([128, QC], BF16, tag="acc",
                                           name="acc")
                        qsl = qT[:, h * S + qc * QC: h * S + (qc + 1) * QC]

                        pend_av = []  # (ex2, ga, diag) awaiting AV matmuls

                        def emit_av(ex2, ga, diag):
                            if not diag:
                                for g, off in ((ga, 0), (ga + 1, QC)):
                                    nc.tensor.matmul(
                                        ps_y[:],
                                        V[:, g * 128:(g + 1) * 128],
                                        ex2[:, off: off + QC],
                                        start=(g == 0), stop=(g == nblk - 1))
                            else:
                                ta = ga - nfull
                                w0 = QC - ta * KB
                                w1 = QC - (ta + 1) * KB
                                nc.tensor.matmul(
                                    ps_y[:, ta * KB:QC],
                                    V[:, ga * 128:(ga + 1) * 128],
                                    ex2[:, 0:w0],
                                    start=(ga == 0), stop=False)
                                nc.tensor.matmul(
                                    ps_y[:, (ta + 1) * KB:QC],
                                    V[:, (ga + 1) * 128:(ga + 2) * 128],
                                    ex2[:, w0:w0 + w1],
                                    start=False, stop=(ga + 1 == nblk - 1))

                        # full (unmasked) kv-block pairs
                        for p in range(nfull // 2):
                            ga = 2 * p
                            pa2 = pa.tile([128, 2 * QC], F32, tag="s",
                                          name="pa2")
                            nc.tensor.matmul(
                                pa2[:, 0:QC],
                                kT[:, ga * KB:(ga + 1) * KB],
                                qsl, start=True, stop=True)
                            nc.tensor.matmul(
                                pa2[:, QC:2 * QC],
                                kT[:, (ga + 1) * KB:(ga + 2) * KB],
                                qsl, start=True, stop=True)
                            ex2 = epool.tile([128, 2 * QC], BF16, tag="e",
                                             name="ex2")
                            nc.scalar.activation(
                                ex2[:], pa2[:],
                                mybir.ActivationFunctionType.Exp, bias=b0[:])
                            # denominator accumulation on DVE (bf16)
                            if p == 0:
                                nc.vector.tensor_add(
                                    acc[:], ex2[:, 0:QC], ex2[:, QC:2 * QC])
                            else:
                                ap = epool.tile([128, QC], BF16, tag="ap",
                                                name="accp")
                                nc.vector.tensor_add(
                                    ap[:], ex2[:, 0:QC], ex2[:, QC:2 * QC])
                                nc.vector.tensor_add(acc[:], acc[:], ap[:])
                            pend_av.append((ex2, ga, False))
                            if len(pend_av) > AVDEPTH:
                                emit_av(*pend_av.pop(0))

                        # diagonal pairs: column-trimmed, left-shifted scores
                        for dp_i in range(2):
                            ta0 = 2 * dp_i
                            ta1 = ta0 + 1
                            ga = nfull + ta0
                            w0 = QC - ta0 * KB
                            w1 = QC - ta1 * KB
                            pa2 = pa.tile([128, 2 * QC], F32, tag="s",
                                          name="pa2d")
                            nc.tensor.matmul(
                                pa2[:, 0:w0],
                                kT[:, ga * KB:(ga + 1) * KB],
                                qsl[:, ta0 * KB:QC], start=True, stop=True)
                            nc.tensor.matmul(
                                pa2[:, w0:w0 + w1],
                                kT[:, (ga + 1) * KB:(ga + 2) * KB],
                                qsl[:, ta1 * KB:QC], start=True, stop=True)
                            ex2 = epool.tile([128, 2 * QC], BF16, tag="e",
                                             name="ex2d")
                            nc.scalar.activation(
                                ex2[:, 0:w0 + w1], pa2[:, 0:w0 + w1],
                                mybir.ActivationFunctionType.Exp, bias=b0[:])
                            # causal staircase bands (one shared triangle)
                            nc.vector.tensor_mul(
                                ex2[:, 0:KB], ex2[:, 0:KB], tri[:])
                            nc.vector.tensor_mul(
                                ex2[:, w0:w0 + KB], ex2[:, w0:w0 + KB],
                                tri[:])
                            # denominator accumulation (aligned slices)
                            if qc == 0 and dp_i == 0:
                                nc.vector.tensor_copy(acc[:], ex2[:, 0:w0])
                            else:
                                nc.vector.tensor_add(
                                    acc[:, ta0 * KB:QC],
                                    acc[:, ta0 * KB:QC], ex2[:, 0:w0])
                            nc.vector.tensor_add(
                                acc[:, ta1 * KB:QC],
                                acc[:, ta1 * KB:QC], ex2[:, w0:w0 + w1])
                            pend_av.append((ex2, ga, True))
                            if len(pend_av) > AVDEPTH:
                                emit_av(*pend_av.pop(0))
                        for args in pend_av:
                            emit_av(*args)

                        # denominator: ones-matmul over the bf16 accumulator;
                        # reciprocal broadcast back to 128 partitions via a
                        # K=1 matmul.  den has its own bank; the broadcast
                        # rides the ps_y ring (its slot frees exactly when
                        # the previous chunk's normalize completes).
                        den = pd.tile([1, QC], F32, tag="den", name="den")
                        nc.tensor.matmul(den[:], ones[:], acc[:],
                                         start=True, stop=True)
                        rec1 = rspool.tile([1, QC], F32, tag="rc1",
                                           name="rec1")
                        nc.vector.reciprocal_approx_fast(rec1[:], den[:])
                        rc16 = rspool.tile([1, QC], BF16, tag="rc6",
                                           name="rc16")
                        nc.vector.tensor_copy(rc16[:], rec1[:])
                        rect = py.tile([128, QC], F32, tag="y", name="rect")
                        rec = rect[:, 0:QC]
                        nc.tensor.matmul(rec, onec[:], rc16[:],
                                         start=True, stop=True)
                        rsb = rspool.tile([128, QC], F32, tag="rsb",
                                          name="rsb")
                        nc.vector.tensor_copy(rsb[:], rec)
                        nc.vector.tensor_mul(
                            yT[:, qc * QC:(qc + 1) * QC], ps_y[:], rsb[:])
                        # stage this token-quarter into both batch halves of
                        # the AllToAll input
                        nc.sync.dma_start(
                            in_b[qc * 128:(qc + 1) * 128, :],
                            yT[:, qc * QC:(qc + 1) * QC])
                        nc.sync.dma_start(
                            in_b[(4 + qc) * 128:(5 + qc) * 128, :],
                            yT[:, qc * QC:(qc + 1) * QC])

                    if debug_taps:
                        nc.sync.dma_start(dbg_y[:, h * S:(h + 1) * S], yT[:])

                    # per-head 8-rank AllToAll re-shard (head- -> token-split)
                    out_b = dpool.tile([8 * 128, TPT], BF16, tag=f"a2o{h}",
                                       name=f"a2aout{h}")
                    nc.gpsimd.collective_compute(
                        "AllToAll", mybir.AluOpType.bypass,
                        replica_groups=RG8,
                        ins=[in_b.opt()], outs=[out_b.opt()])
                    # readback queued behind the A2A on the gpsimd queue:
                    # fires the moment the collective completes.  Chunks are
                    # permuted so panel p holds sender (rk+p)%4 — matching
                    # the host-side wo panel permutation and making panel 0
                    # always this rank's own head.
                    for p in range(4):
                        row = nc.s_assert_within(
                            (pid - pid % 4 + (pid % 4 + p) % 4) * 128,
                            0, 896, skip_runtime_assert=True)
                        nc.gpsimd.dma_start(
                            yf_h[h][:, p * TPT:(p + 1) * TPT],
                            out_b[bass.ds(row, 128), :])

                    if debug_taps:
                        nc.sync.dma_start(
                            dbg_yf[:, h * 4 * TPT:(h + 1) * 4 * TPT],
                            yf_h[h][:])

                    if h == 3:
                        yT3 = yT

                # attention done: swap the attention PSUM pools for a deep
                # outproj ring.  All ops run AFTER attention — by then every
                # re-shard except head 3's has landed with tens of us of
                # slack (robust to rank skew), and head 3's own-rank quarter
                # comes straight from local yT while its AllToAll flies.
                pd_cm.__exit__(None, None, None)
                py_cm.__exit__(None, None, None)
                attn_psum.__exit__(None, None, None)
                po_cm = tc.tile_pool(name="po", bufs=6, space="PSUM")
                op_pool[0] = po_cm.__enter__()
                for ot in range(NDT):
                    op01(ot)
                for ot in range(NDT):
                    op2_3self(ot, yT3)
                for ot in range(NDT):
                    op3rest(ot)
                po_cm.__exit__(None, None, None)

    nc.compile()
    return nc


def make_in_maps(x, freqs_cis, wqkv, wo, q_norm_w, k_norm_w, S):
    """Host-side sharding / layout prep. Returns list of 8 input dicts."""
    bf = ml_dtypes.bfloat16
    KB = 128

    # rope tables: [128, S]; row 2i & 2i+1 carry cos[t, i]; sin signed
    cos = np.asarray(freqs_cis[:S, :, 0], np.float32)   # [S, 64]
    sin = np.asarray(freqs_cis[:S, :, 1], np.float32)
    cosF = np.ascontiguousarray(np.repeat(cos.T, 2, axis=0)).astype(bf)
    sinF = np.repeat(sin.T, 2, axis=0).astype(np.float32)
    sinF[0::2] *= -1.0
    sinF = np.ascontiguousarray(sinF).astype(bf)

    ident = np.eye(128, dtype=bf)

    # shared causal staircase triangle: allowed iff kv-row r <= stored col p
    r = np.arange(KB)[:, None]
    p = np.arange(KB)[None, :]
    tri = (r <= p).astype(np.float32).astype(bf)

    qwv = np.asarray(q_norm_w, np.float32).reshape(128, 1)
    kwv = np.asarray(k_norm_w, np.float32).reshape(128, 1)

    woT = np.ascontiguousarray(np.asarray(wo, np.float32).T).astype(bf)
    # per-core panel permutation: row-block (4p+g) holds the wo columns of
    # head 4*((rk+p)%4)+g, so in-kernel panel p is sender (rk+p)%4 and
    # panel 0 is always the core's own head
    woT_core = []
    for rk in range(4):
        wc = np.empty_like(woT)
        for p in range(4):
            for g in range(4):
                src = 4 * ((rk + p) % 4) + g
                dst = 4 * p + g
                wc[dst * 128:(dst + 1) * 128] = woT[src * 128:(src + 1) * 128]
        woT_core.append(np.ascontiguousarray(wc))

    xTb = []
    for b in range(2):
        xTb.append(np.ascontiguousarray(np.asarray(x[b], np.float32).T)
                   .astype(bf))

    wq = np.asarray(wqkv, np.float32)
    q_sz = NH * HD
    in_maps = []
    for c_id in range(N_CORES):
        b, rk = c_id // 4, c_id % 4
        rows = np.concatenate([
            wq[rk * HL * HD:(rk + 1) * HL * HD],          # 4 q heads
            wq[q_sz + rk * HD: q_sz + (rk + 1) * HD],     # k head
            wq[q_sz + NKV * HD + rk * HD:
               q_sz + NKV * HD + (rk + 1) * HD],          # v head
        ], axis=0)                                        # [768, 2048]
        wslice = np.ascontiguousarray(rows.T).astype(bf)  # [2048, 768]
        in_maps.append({
            "xT": xTb[b], "wslice": wslice, "woT": woT_core[rk],
            "cosF": cosF, "sinF": sinF,
            "ident": ident, "tri": tri,
            "qw": qwv, "kw": kwv,
        })
    return in_maps


_NC_CACHE = {}


def kernel(x, freqs_cis, mask, wqkv, wo, q_norm_w, k_norm_w):
    x = np.asarray(x)
    S = x.shape[1]
    if S not in _NC_CACHE:
        _NC_CACHE[S] = build_graph(S)
    nc = _NC_CACHE[S]
    in_maps = make_in_maps(x, freqs_cis, wqkv, wo, q_norm_w, k_norm_w, S)
    res = run_bass_kernel_spmd(nc, in_maps, core_ids=list(range(N_CORES)))
    TPT = S // 4
    out = np.empty((2, S, DIM), np.float32)
    for c_id in range(N_CORES):
        b, rk = c_id // 4, c_id % 4
        out[b, rk * TPT:(rk + 1) * TPT, :] = res.results[c_id]["out"].T.astype(np.float32)
    return out


# revision 42
# speedup vs baseline: 1.0755x; 1.0646x over previous
"""Distributed Trainium2 kernel for nn_Attention (dense transformer block:
fused QKV projection + per-head RMSNorm + rotary + causal GQA attention + output
projection), running SPMD on 8 NeuronCores.

Sharding (rank-uniform, no divergent control flow):
  - 8 cores = 2 batch groups x 4 tensor-parallel ranks.
  - Core c: batch b = c // 4, rank r = c % 4.
  - QKV projection + attention are head-sharded: core r computes q heads
    4r..4r+3 and kv head r for ALL tokens of its batch.
  - One 8-rank AllToAll per head re-shards y from head-split to token-split
    (each rank stages its y token-quarters in both batch halves' chunk slots;
    receivers read only their own batch's half).  A2A moves 4x less wire
    data than the AllGather it replaces and runs ~13us vs ~42us.
  - The output projection for head h-1 is interleaved after head h's
    attention, so its matmuls fill the tensor-engine slack while the
    scalar engine (exp) paces attention.

Layout tricks:
  - Host pre-transposes x, wqkv, wo so the kernel's matmuls need no on-device
    transposes (except tiny 128x128 PE transposes for V).
  - Rope's even/odd pair swap is a 128x128 permutation matmul in bf16.
  - Scores are computed transposed [kv, q]; diagonal kv-blocks are column-
    trimmed (fully-masked q columns never computed), stored left-shifted so
    each diagonal pair still needs only ONE exp activation; the remaining
    128-wide causal staircase band uses a single shared triangular mask.
  - exp is fused into the PSUM->SBUF eviction on the ScalarEngine, batched
    two kv-blocks per activation to amortize the ~300-cycle overhead.
  - The softmax denominator is accumulated on the VectorEngine in bf16 and
    reduced across partitions with a ones-vector matmul; the reciprocal is
    broadcast back to 128 partitions with a K=1 matmul.  Both matmul
    outputs live in slices of the score-pair PSUM ring, freeing two PSUM
    banks for the interleaved output projection.
  - RMSNorm reduces to a per-token scalar via a ones-vector matmul over the
    squared tile; the 1/sqrt(head_dim) score scale folds into the q-side
    scalar.
  - All big matmuls run in bf16 with f32 PSUM accumulation.
  - x is staged through a 2-deep ring of per-token-chunk tiles in need
    order; wo panels prefetch during phase-1's idle DMA tail.
"""

import numpy as np
import ml_dtypes

import concourse.bass as bass
import concourse.mybir as mybir
import concourse.tile as tile
from concourse import bacc
from concourse.bass_utils import run_bass_kernel_spmd

BF16 = mybir.dt.bfloat16
F32 = mybir.dt.float32

DIM = 2048
NH = 16
NKV = 4
HD = 128
EPS = 1e-5
N_CORES = 8
RG8 = [[0, 1, 2, 3, 4, 5, 6, 7]]  # 8-rank group for the AllToAll

HL = NH // NKV  # q heads per core (= GQA group size) = 4
EW = HL * HD + 2 * HD  # wqkv column-slice width per core = 768
NDT = DIM // 128  # contraction tiles = 16
SWAP_MASK = [i ^ 1 for i in range(32)]  # rope pair swap, per 32-lane group


def build_graph(S, debug_taps=False):
    """Build + compile the SPMD graph for sequence length S. Returns nc."""
    TPT = S // 4       # tokens per core after the re-shard (output rows)
    TCW = S // 4       # token chunk width for phase 1 (moving dim <= 512)
    NTT = S // TCW     # number of token chunks = 4
    QC = 512           # attention q-chunk width
    KB = 128           # kv block size
    NQC = S // QC      # q chunks per head
    AVDEPTH = 3        # kv-block pairs the AV matmuls trail the score matmuls

    nc = bacc.Bacc("TRN2", target_bir_lowering=False, debug=False,
                   num_devices=N_CORES)

    # ---- DRAM I/O ----
    xT_d = nc.dram_tensor("xT", [DIM, S], BF16, kind="ExternalInput")
    w_d = nc.dram_tensor("wslice", [DIM, EW], BF16, kind="ExternalInput")
    wo_d = nc.dram_tensor("woT", [DIM, DIM], BF16, kind="ExternalInput")
    cos_d = nc.dram_tensor("cosF", [128, S], BF16, kind="ExternalInput")
    sin_d = nc.dram_tensor("sinF", [128, S], BF16, kind="ExternalInput")
    idn_d = nc.dram_tensor("ident", [128, 128], BF16, kind="ExternalInput")
    tri_d = nc.dram_tensor("tri", [KB, KB], BF16, kind="ExternalInput")
    qw_d = nc.dram_tensor("qw", [128, 1], F32, kind="ExternalInput")
    kw_d = nc.dram_tensor("kw", [128, 1], F32, kind="ExternalInput")
    out_d = nc.dram_tensor("out", [DIM, TPT], BF16, kind="ExternalOutput")
    if debug_taps:
        dbg_q = nc.dram_tensor("dbg_q", [128, HL * S], BF16,
                               kind="ExternalOutput")
        dbg_k = nc.dram_tensor("dbg_k", [128, S], BF16, kind="ExternalOutput")
        dbg_v = nc.dram_tensor("dbg_v", [128, S], BF16, kind="ExternalOutput")
        dbg_y = nc.dram_tensor("dbg_y", [128, HL * S], BF16,
                               kind="ExternalOutput")
        dbg_yf = nc.dram_tensor("dbg_yf", [128, HL * 4 * TPT], BF16,
                                kind="ExternalOutput")

    with tile.TileContext(nc) as tc:
        with tc.tile_pool(name="const", bufs=1) as cpool, \
             tc.tile_pool(name="big", bufs=1) as bigpool, \
             tc.tile_pool(name="wo", bufs=1) as wopool, \
             tc.tile_pool(name="exp", bufs=6) as epool, \
             tc.tile_pool(name="acc", bufs=2) as accpool, \
             tc.tile_pool(name="yt", bufs=2) as ytpool, \
             tc.tile_pool(name="dram", bufs=1, space="DRAM") as dpool:

            # constants (gpsimd queue: keep the sync queue free for weights)
            idn = cpool.tile([128, 128], BF16, tag="idn")
            nc.gpsimd.dma_start(idn[:], idn_d[:])
            tri = cpool.tile([KB, KB], BF16, tag="tri")
            nc.gpsimd.dma_start(tri[:], tri_d[:])
            qw = cpool.tile([128, 1], F32, tag="qw")
            nc.gpsimd.dma_start(qw[:], qw_d[:])
            kw = cpool.tile([128, 1], F32, tag="kw")
            nc.gpsimd.dma_start(kw[:], kw_d[:])
            ones = cpool.tile([128, 1], BF16, tag="ones")
            nc.vector.memset(ones[:], 1.0)
            onec = cpool.tile([1, 128], BF16, tag="onec")
            nc.vector.memset(onec[:], 1.0)
            b0 = cpool.tile([128, 1], F32, tag="b0")
            nc.vector.memset(b0[:], 0.0)
            bq = cpool.tile([1, 1], F32, tag="bq")
            nc.vector.memset(bq[:], float(HD * EPS))
            bk = cpool.tile([1, 1], F32, tag="bk")
            nc.vector.memset(bk[:], float(EPS))

            # long-lived activations
            qT = bigpool.tile([128, HL * S], BF16, tag="qT")
            kT = bigpool.tile([128, S], BF16, tag="kT")
            V = bigpool.tile([128, S], BF16, tag="V")   # [tok%128, blk*128+d]

            # output-projection weight panels; the DMAs are issued during
            # phase 1 so they ride its idle DMA tail
            wo_h = [wopool.tile([128, 4 * S], BF16, tag=f"wo{h}",
                                name=f"wo{h}")
                    for h in range(HL)]

            # tiny warm-up AllToAll: pays the collective firmware's cold
            # start in the shadow of phase-1 compute so the real per-head
            # re-shards fire promptly
            wu_in = dpool.tile([8 * 16, 16], BF16, tag="wuin", name="wu_in")
            wu_out = dpool.tile([8 * 16, 16], BF16, tag="wuout",
                                name="wu_out")
            nc.gpsimd.collective_compute(
                "AllToAll", mybir.AluOpType.bypass, replica_groups=RG8,
                ins=[wu_in.opt()], outs=[wu_out.opt()])

            # ---------------- Phase 1: QKV + norm + rope ----------------
            with tc.tile_pool(name="wq", bufs=1) as wpool, \
                 tc.tile_pool(name="x", bufs=2) as xpool, \
                 tc.tile_pool(name="cs", bufs=2) as cspool, \
                 tc.tile_pool(name="scr", bufs=2) as scr, \
                 tc.tile_pool(name="smol", bufs=2) as smol, \
                 tc.tile_pool(name="p1", bufs=5, space="PSUM") as p1, \
                 tc.tile_pool(name="pss", bufs=1, space="PSUM") as pss, \
                 tc.tile_pool(name="pvt", bufs=2, space="PSUM") as pvt:

                # full wqkv slice, staged once: [128, dt*EW + e]
                w_sb = wpool.tile([128, NDT * EW], BF16, tag="w")

                xr = [None] * NTT

                def issue_x(tt):
                    xr[tt] = xpool.tile([128, NDT * TCW], BF16, tag="x",
                                        name=f"x{tt}")
                    for dt in range(NDT):
                        # tt0 rides the scalar queue alone so it is never
                        # queued behind the 3MB of weights on sync
                        eng = (nc.scalar if (tt == 0 or dt % 2 == 0)
                               else nc.sync)
                        eng.dma_start(
                            xr[tt][:, dt * TCW:(dt + 1) * TCW],
                            xT_d[dt * 128:(dt + 1) * 128,
                                 tt * TCW:(tt + 1) * TCW])

                def process_qk(ps, et, tt, cos_t, sin_t):
                    is_q = et < HL
                    # sum of squares over head_dim via ones-vector matmul
                    sqv = smol.tile([128, TCW], BF16, tag="sq2", name="sqv")
                    nc.scalar.activation(
                        sqv[:], ps[:],
                        mybir.ActivationFunctionType.Square, bias=b0[:])
                    ss = pss.tile([1, TCW], F32, tag="ss", name="ss")
                    nc.tensor.matmul(ss[:], ones[:], sqv[:],
                                     start=True, stop=True)
                    sq = smol.tile([1, TCW], F32, tag="sqs", name="sq")
                    if is_q:
                        # 1/sqrt(ss + HD*eps) folds the 1/sqrt(HD) score scale
                        nc.scalar.activation(
                            sq[:], ss[:],
                            mybir.ActivationFunctionType.Sqrt,
                            bias=bq[:], scale=1.0)
                    else:
                        nc.scalar.activation(
                            sq[:], ss[:],
                            mybir.ActivationFunctionType.Sqrt,
                            bias=bk[:], scale=1.0 / HD)
                    inv = smol.tile([1, TCW], F32, tag="inv", name="inv")
                    nc.vector.reciprocal_approx_fast(inv[:], sq[:])
                    invb = scr.tile([128, TCW], F32, tag="invb", name="invb")
                    nc.gpsimd.partition_broadcast(invb[:], inv[:])
                    qf = scr.tile([128, TCW], BF16, tag="qf", name="qf")
                    nc.scalar.mul(qf[:], ps[:], (qw if is_q else kw)[:])
                    # rope: pair swap on the DVE lane shuffler, sinF signed
                    sw = scr.tile([128, TCW], BF16, tag="sw", name="sw")
                    nc.vector.stream_shuffle(sw[:], qf[:], SWAP_MASK)
                    t1 = scr.tile([128, TCW], F32, tag="t1", name="t1")
                    nc.vector.tensor_mul(t1[:], qf[:], cos_t[:])
                    t2 = scr.tile([128, TCW], F32, tag="t2", name="t2")
                    nc.vector.tensor_mul(t2[:], sw[:], sin_t[:])
                    nc.vector.tensor_add(t1[:], t1[:], t2[:])
                    dst = (qT[:, et * S + tt * TCW: et * S + tt * TCW + TCW]
                           if is_q else
                           kT[:, tt * TCW: tt * TCW + TCW])
                    nc.vector.tensor_mul(dst, t1[:], invb[:])

                def process_v(ps, tt):
                    vb = smol.tile([128, TCW], BF16, tag="vb", name="vb")
                    nc.scalar.copy(vb[:], ps[:])
                    for bb in range(TCW // 128):
                        tp = pvt.tile([128, 128], BF16, tag="tp", name="tp")
                        nc.tensor.transpose(
                            tp[:], vb[:, bb * 128:(bb + 1) * 128], idn[:])
                        blk = tt * (TCW // 128) + bb
                        nc.scalar.copy(V[:, blk * 128:(blk + 1) * 128], tp[:])

                # weights + first token chunk in need-order
                for dt in range(NDT):
                    nc.sync.dma_start(
                        w_sb[:, dt * EW:(dt + 1) * EW],
                        w_d[dt * 128:(dt + 1) * 128, :])
                issue_x(0)

                pend = []  # (psum, et, tt, cos_t, sin_t) awaiting processing

                def process_one():
                    pps, pet, ptt, pc, psn_ = pend.pop(0)
                    if pet < HL + 1:
                        process_qk(pps, pet, ptt, pc, psn_)
                    else:
                        process_v(pps, ptt)

                for tt in range(NTT):
                    cos_t = cspool.tile([128, TCW], BF16, tag="cos")
                    nc.sync.dma_start(cos_t[:], cos_d[:, tt * TCW:(tt + 1) * TCW])
                    sin_t = cspool.tile([128, TCW], BF16, tag="sin")
                    nc.sync.dma_start(sin_t[:], sin_d[:, tt * TCW:(tt + 1) * TCW])
                    if tt + 1 < NTT:
                        issue_x(tt + 1)
                    if tt == NTT - 1:
                        # wo prefetch rides phase-1's idle DMA tail
                        for h in range(HL):
                            for j in range(4):
                                et = 4 * j + h
                                nc.sync.dma_start(
                                    wo_h[h][:, j * S:(j + 1) * S],
                                    wo_d[et * 128:(et + 1) * 128, :])

                    # k and v first: attention's inputs finish earliest and
                    # the end-of-phase drain holds only late q heads
                    ets = [HL, HL + 1] + list(range(HL))
                    first_ei = 0
                    if tt == 0:
                        # dt-outer warm-up triple: the matmul stream paces
                        # the (w[dt], x[dt]) DMA trickle instead of stalling
                        # on the full 5MB prefix before the first group
                        first_ei = 3
                        trip = [p1.tile([128, TCW], F32, tag="ps",
                                        name=f"ps_w{ei}") for ei in range(3)]
                        for dt in range(NDT):
                            for ei in range(3):
                                et = ets[ei]
                                nc.tensor.matmul(
                                    trip[ei][:],
                                    w_sb[:, dt * EW + et * 128:
                                         dt * EW + (et + 1) * 128],
                                    xr[0][:, dt * TCW:(dt + 1) * TCW],
                                    start=(dt == 0), stop=(dt == NDT - 1),
                                )
                        for ei in range(3):
                            pend.append((trip[ei], ets[ei], 0, cos_t, sin_t))

                    for et in ets[first_ei:]:
                        ps = p1.tile([128, TCW], F32, tag="ps")
                        for dt in range(NDT):
                            nc.tensor.matmul(
                                ps[:],
                                w_sb[:, dt * EW + et * 128:dt * EW + (et + 1) * 128],
                                xr[tt][:, dt * TCW:(dt + 1) * TCW],
                                start=(dt == 0), stop=(dt == NDT - 1),
                            )
                        # process an older tile now: its cross-engine waits
                        # overlap this tile's matmul group
                        if pend:
                            process_one()
                        if tt == NTT - 1 and pend:
                            process_one()  # eager drain: shallow phase exit
                        pend.append((ps, et, tt, cos_t, sin_t))
                while pend:
                    process_one()

            if debug_taps:
                nc.sync.dma_start(dbg_q[:], qT[:])
                nc.sync.dma_start(dbg_k[:], kT[:])
                nc.sync.dma_start(dbg_v[:], V[:])

            # ------- Phase 2: causal attention, then output projection -----
            with tc.tile_pool(name="part", bufs=1) as partpool, \
                 tc.tile_pool(name="yf", bufs=1) as yfpool, \
                 tc.tile_pool(name="rs", bufs=2) as rspool, \
                 tc.tile_pool(name="ot", bufs=2) as otpool:

                part = partpool.tile([128, NDT * TPT], F32, tag="part")
                yf_h = [yfpool.tile([128, 4 * TPT], BF16, tag=f"yf{h}",
                                    name=f"yf{h}")
                        for h in range(HL)]
                pid = nc.gpsimd.partition_id()
                # token-quarter base of this rank, on the PE register file
                # (it feeds a matmul moving-operand offset)
                pid_pe = nc.tensor.partition_id()
                roff = nc.s_assert_within((pid_pe % 4) * TPT, 0, S - TPT,
                                          skip_runtime_assert=True)
                op_pool = [None]

                def op_tile():
                    ps_o = op_pool[0].tile([128, TPT], F32, tag="o",
                                           name="ps_o")
                    return ps_o[:]

                def op01(ot):
                    # heads 0+1 accumulate in one PSUM group: 1 evict per ot
                    ps_o = op_tile()
                    for g in (0, 1):
                        for p in range(4):
                            nc.tensor.matmul(
                                ps_o,
                                wo_h[g][:, p * S + ot * 128: p * S + ot * 128 + 128],
                                yf_h[g][:, p * TPT:(p + 1) * TPT],
                                start=(g == 0 and p == 0),
                                stop=(g == 1 and p == 3))
                    # evict on ScalarE: DVE is the busier engine mid-window
                    nc.scalar.copy(part[:, ot * TPT:(ot + 1) * TPT], ps_o)

                def op2_3self(ot, yT3):
                    # head 2 (re-shard done long ago) + head 3's own-rank
                    # quarter straight out of local yT: runs BEFORE the last
                    # AllToAll lands
                    ps_o = op_tile()
                    for p in range(4):
                        nc.tensor.matmul(
                            ps_o,
                            wo_h[2][:, p * S + ot * 128: p * S + ot * 128 + 128],
                            yf_h[2][:, p * TPT:(p + 1) * TPT],
                            start=(p == 0), stop=False)
                    nc.tensor.matmul(
                        ps_o,
                        wo_h[3][:, 0 * S + ot * 128: 0 * S + ot * 128 + 128],
                        yT3[:, bass.ds(roff, TPT)],
                        start=False, stop=True)
                    psl = part[:, ot * TPT:(ot + 1) * TPT]
                    nc.vector.tensor_add(psl, psl, ps_o)

                def op3rest(ot):
                    # head 3's three remote quarters: the only work gated on
                    # the final AllToAll
                    ps_o = op_tile()
                    for p in (1, 2, 3):
                        nc.tensor.matmul(
                            ps_o,
                            wo_h[3][:, p * S + ot * 128: p * S + ot * 128 + 128],
                            yf_h[3][:, p * TPT:(p + 1) * TPT],
                            start=(p == 1), stop=(p == 3))
                    ott = otpool.tile([128, TPT], BF16, tag="ot", name="ott")
                    nc.vector.tensor_add(
                        ott[:], ps_o, part[:, ot * TPT:(ot + 1) * TPT])
                    nc.sync.dma_start(out_d[ot * 128:(ot + 1) * 128, :],
                                      ott[:])

                attn_psum = tc.tile_pool(name="pa", bufs=2, space="PSUM")
                pa = attn_psum.__enter__()
                py_cm = tc.tile_pool(name="py", bufs=2, space="PSUM")
                py = py_cm.__enter__()
                pd_cm = tc.tile_pool(name="pd", bufs=2, space="PSUM")
                pd = pd_cm.__enter__()

                for h in range(HL):
                    yT = ytpool.tile([128, S], BF16, tag="yT", name="yT")
                    in_b = dpool.tile([8 * 128, TPT], BF16, tag=f"a2i{h}",
                                      name=f"a2ain{h}")
                    for qc in range(NQC):
                        nblk = 4 * (qc + 1)
                        nfull = 4 * qc
                        ps_y = py.tile([128, QC], F32, tag="y", name="ps_y")
                        acc = accpool.tile([128, QC], BF16, tag="acc",
                                           name="acc")
                        qsl = qT[:, h * S + qc * QC: h * S + (qc + 1) * QC]

                        pend_av = []  # (ex2, ga, diag) awaiting AV matmuls

                        def emit_av(ex2, ga, diag):
                            if not diag:
                                for g, off in ((ga, 0), (ga + 1, QC)):
                                    nc.tensor.matmul(
                                        ps_y[:],
                                        V[:, g * 128:(g + 1) * 128],
                                        ex2[:, off: off + QC],
                                        start=(g == 0), stop=(g == nblk - 1))
                            else:
                                ta = ga - nfull
                                w0 = QC - ta * KB
                                w1 = QC - (ta + 1) * KB
                                nc.tensor.matmul(
                                    ps_y[:, ta * KB:QC],
                                    V[:, ga * 128:(ga + 1) * 128],
                                    ex2[:, 0:w0],
                                    start=(ga == 0), stop=False)
                                nc.tensor.matmul(
                                    ps_y[:, (ta + 1) * KB:QC],
                                    V[:, (ga + 1) * 128:(ga + 2) * 128],
                                    ex2[:, w0:w0 + w1],
                                    start=False, stop=(ga + 1 == nblk - 1))

                        # full (unmasked) kv-block pairs
                        for p in range(nfull // 2):
                            ga = 2 * p
                            pa2 = pa.tile([128, 2 * QC], F32, tag="s",
                                          name="pa2")
                            nc.tensor.matmul(
                                pa2[:, 0:QC],
                                kT[:, ga * KB:(ga + 1) * KB],
                                qsl, start=True, stop=True)
                            nc.tensor.matmul(
                                pa2[:, QC:2 * QC],
                                kT[:, (ga + 1) * KB:(ga + 2) * KB],
                                qsl, start=True, stop=True)
                            ex2 = epool.tile([128, 2 * QC], BF16, tag="e",
                                             name="ex2")
                            nc.scalar.activation(
                                ex2[:], pa2[:],
                                mybir.ActivationFunctionType.Exp, bias=b0[:])
                            # denominator accumulation on DVE (bf16)
                            if p == 0:
                                nc.vector.tensor_add(
                                    acc[:], ex2[:, 0:QC], ex2[:, QC:2 * QC])
                            else:
                                ap = epool.tile([128, QC], BF16, tag="ap",
                                                name="accp")
                                nc.vector.tensor_add(
                                    ap[:], ex2[:, 0:QC], ex2[:, QC:2 * QC])
                                nc.vector.tensor_add(acc[:], acc[:], ap[:])
                            pend_av.append((ex2, ga, False))
                            if len(pend_av) > AVDEPTH:
                                emit_av(*pend_av.pop(0))

                        # diagonal pairs: column-trimmed, left-shifted scores
                        for dp_i in range(2):
                            ta0 = 2 * dp_i
                            ta1 = ta0 + 1
                            ga = nfull + ta0
                            w0 = QC - ta0 * KB
                            w1 = QC - ta1 * KB
                            pa2 = pa.tile([128, 2 * QC], F32, tag="s",
                                          name="pa2d")
                            nc.tensor.matmul(
                                pa2[:, 0:w0],
                                kT[:, ga * KB:(ga + 1) * KB],
                                qsl[:, ta0 * KB:QC], start=True, stop=True)
                            nc.tensor.matmul(
                                pa2[:, w0:w0 + w1],
                                kT[:, (ga + 1) * KB:(ga + 2) * KB],
                                qsl[:, ta1 * KB:QC], start=True, stop=True)
                            ex2 = epool.tile([128, 2 * QC], BF16, tag="e",
                                             name="ex2d")
                            nc.scalar.activation(
                                ex2[:, 0:w0 + w1], pa2[:, 0:w0 + w1],
                                mybir.ActivationFunctionType.Exp, bias=b0[:])
                            # causal staircase bands (one shared triangle)
                            nc.vector.tensor_mul(
                                ex2[:, 0:KB], ex2[:, 0:KB], tri[:])
                            nc.vector.tensor_mul(
                                ex2[:, w0:w0 + KB], ex2[:, w0:w0 + KB],
                                tri[:])
                            # denominator accumulation (aligned slices)
                            if qc == 0 and dp_i == 0:
                                nc.vector.tensor_copy(acc[:], ex2[:, 0:w0])
                            else:
                                nc.vector.tensor_add(
                                    acc[:, ta0 * KB:QC],
                                    acc[:, ta0 * KB:QC], ex2[:, 0:w0])
                            nc.vector.tensor_add(
                                acc[:, ta1 * KB:QC],
                                acc[:, ta1 * KB:QC], ex2[:, w0:w0 + w1])
                            pend_av.append((ex2, ga, True))
                            if len(pend_av) > AVDEPTH:
                                emit_av(*pend_av.pop(0))
                        for args in pend_av:
                            emit_av(*args)

                        # denominator: ones-matmul over the bf16 accumulator;
                        # reciprocal broadcast back to 128 partitions via a
                        # K=1 matmul.  den has its own bank; the broadcast
                        # rides the ps_y ring (its slot frees exactly when
                        # the previous chunk's normalize completes).
                        den = pd.tile([1, QC], F32, tag="den", name="den")
                        nc.tensor.matmul(den[:], ones[:], acc[:],
                                         start=True, stop=True)
                        rec1 = rspool.tile([1, QC], F32, tag="rc1",
                                           name="rec1")
                        nc.vector.reciprocal_approx_fast(rec1[:], den[:])
                        rc16 = rspool.tile([1, QC], BF16, tag="rc6",
                                           name="rc16")
                        nc.vector.tensor_copy(rc16[:], rec1[:])
                        rect = py.tile([128, QC], F32, tag="y", name="rect")
                        rec = rect[:, 0:QC]
                        nc.tensor.matmul(rec, onec[:], rc16[:],
                                         start=True, stop=True)
                        rsb = rspool.tile([128, QC], F32, tag="rsb",
                                          name="rsb")
                        nc.vector.tensor_copy(rsb[:], rec)
                        nc.vector.tensor_mul(
                            yT[:, qc * QC:(qc + 1) * QC], ps_y[:], rsb[:])
                        # stage this token-quarter into both batch halves of
                        # the AllToAll input
                        nc.sync.dma_start(
                            in_b[qc * 128:(qc + 1) * 128, :],
                            yT[:, qc * QC:(qc + 1) * QC])
                        nc.sync.dma_start(
                            in_b[(4 + qc) * 128:(5 + qc) * 128, :],
                            yT[:, qc * QC:(qc + 1) * QC])

                    if debug_taps:
                        nc.sync.dma_start(dbg_y[:, h * S:(h + 1) * S], yT[:])

                    # per-head 8-rank AllToAll re-shard (head- -> token-split)
                    out_b = dpool.tile([8 * 128, TPT], BF16, tag=f"a2o{h}",
                                       name=f"a2aout{h}")
                    nc.gpsimd.collective_compute(
                        "AllToAll", mybir.AluOpType.bypass,
                        replica_groups=RG8,
                        ins=[in_b.opt()], outs=[out_b.opt()])
                    # readback queued behind the A2A on the gpsimd queue:
                    # fires the moment the collective completes.  Chunks are
                    # permuted so panel p holds sender (rk+p)%4 — matching
                    # the host-side wo panel permutation and making panel 0
                    # always this rank's own head.
                    for p in range(4):
                        row = nc.s_assert_within(
                            (pid - pid % 4 + (pid % 4 + p) % 4) * 128,
                            0, 896, skip_runtime_assert=True)
                        nc.gpsimd.dma_start(
                            yf_h[h][:, p * TPT:(p + 1) * TPT],
                            out_b[bass.ds(row, 128), :])

                    if debug_taps:
                        nc.sync.dma_start(
                            dbg_yf[:, h * 4 * TPT:(h + 1) * 4 * TPT],
                            yf_h[h][:])

                    if h == 3:
                        yT3 = yT

                # attention done: swap the attention PSUM pools for a deep
                # outproj ring.  All ops run AFTER attention — by then every
                # re-shard except head 3's has landed with tens of us of
                # slack (robust to rank skew), and head 3's own-rank quarter
                # comes straight from local yT while its AllToAll flies.
                pd_cm.__exit__(None, None, None)
                py_cm.__exit__(None, None, None)
                attn_psum.__exit__(None, None, None)
                po_cm = tc.tile_pool(name="po", bufs=6, space="PSUM")
                op_pool[0] = po_cm.__enter__()
                for ot in range(NDT):
                    op01(ot)
                for ot in range(NDT):
                    op2_3self(ot, yT3)
                for ot in range(NDT):
                    op3rest(ot)
                po_cm.__exit__(None, None, None)

    nc.compile()
    return nc


def make_in_maps(x, freqs_cis, wqkv, wo, q_norm_w, k_norm_w, S):
    """Host-side sharding / layout prep. Returns list of 8 input dicts."""
    bf = ml_dtypes.bfloat16
    KB = 128

    # rope tables: [128, S]; row 2i & 2i+1 carry cos[t, i]; sin signed
    cos = np.asarray(freqs_cis[:S, :, 0], np.float32)   # [S, 64]
    sin = np.asarray(freqs_cis[:S, :, 1], np.float32)
    cosF = np.ascontiguousarray(np.repeat(cos.T, 2, axis=0)).astype(bf)
    sinF = np.repeat(sin.T, 2, axis=0).astype(np.float32)
    sinF[0::2] *= -1.0
    sinF = np.ascontiguousarray(sinF).astype(bf)

    ident = np.eye(128, dtype=bf)

    # shared causal staircase triangle: allowed iff kv-row r <= stored col p
    r = np.arange(KB)[:, None]
    p = np.arange(KB)[None, :]
    tri = (r <= p).astype(np.float32).astype(bf)

    qwv = np.asarray(q_norm_w, np.float32).reshape(128, 1)
    kwv = np.asarray(k_norm_w, np.float32).reshape(128, 1)

    woT = np.ascontiguousarray(np.asarray(wo, np.float32).T).astype(bf)
    # per-core panel permutation: row-block (4p+g) holds the wo columns of
    # head 4*((rk+p)%4)+g, so in-kernel panel p is sender (rk+p)%4 and
    # panel 0 is always the core's own head
    woT_core = []
    for rk in range(4):
        wc = np.empty_like(woT)
        for p in range(4):
            for g in range(4):
                src = 4 * ((rk + p) % 4) + g
                dst = 4 * p + g
                wc[dst * 128:(dst + 1) * 128] = woT[src * 128:(src + 1) * 128]
        woT_core.append(np.ascontiguousarray(wc))

    xTb = []
    for b in range(2):
        xTb.append(np.ascontiguousarray(np.asarray(x[b], np.float32).T)
                   .astype(bf))

    wq = np.asarray(wqkv, np.float32)
    q_sz = NH * HD
    in_maps = []
    for c_id in range(N_CORES):
        b, rk = c_id // 4, c_id % 4
        rows = np.concatenate([
            wq[rk * HL * HD:(rk + 1) * HL * HD],          # 4 q heads
            wq[q_sz + rk * HD: q_sz + (rk + 1) * HD],     # k head
            wq[q_sz + NKV * HD + rk * HD:
               q_sz + NKV * HD + (rk + 1) * HD],          # v head
        ], axis=0)                                        # [768, 2048]
        wslice = np.ascontiguousarray(rows.T).astype(bf)  # [2048, 768]
        in_maps.append({
            "xT": xTb[b], "wslice": wslice, "woT": woT_core[rk],
            "cosF": cosF, "sinF": sinF,
            "ident": ident, "tri": tri,
            "qw": qwv, "kw": kwv,
        })
    return in_maps


_NC_CACHE = {}


def kernel(x, freqs_cis, mask, wqkv, wo, q_norm_w, k_norm_w):
    x = np.asarray(x)
    S = x.shape[1]
    if S not in _NC_CACHE:
        _NC_CACHE[S] = build_graph(S)
    nc = _NC_CACHE[S]
    in_maps = make_in_maps(x, freqs_cis, wqkv, wo, q_norm_w, k_norm_w, S)
    res = run_bass_kernel_spmd(nc, in_maps, core_ids=list(range(N_CORES)))
    TPT = S // 4
    out = np.empty((2, S, DIM), np.float32)
    for c_id in range(N_CORES):
        b, rk = c_id // 4, c_id % 4
        out[b, rk * TPT:(rk + 1) * TPT, :] = res.results[c_id]["out"].T.astype(np.float32)
    return out
